# revision 19
# baseline (speedup 1.0000x reference)
"""JointAngleLoss Trainium2 kernel (8-core data-parallel), v5.

Input : pose23d_pred [524288, 21, 3] float32
Output: scalar float32 loss (matches reference.reference)

Strategy: pure data-parallel over the batch dim; each of 8 NeuronCores handles
65536 rows. Host pre-permutes the input into a per-partition bone layout
BONES[c][q][f][k] (bones differenced on host in fp32, then cast to fp16; loss
tolerance 2e-2, measured end-to-end error ~1e-5), so every device-side vector
operand is a contiguous fp16 slice (DVE 2x_1P packed mode). Shipping bones
(60 values/row) instead of joints (75) cuts both DMA bytes and the on-device
bone-subtraction stage (-22% DVE work vs v4).

Groups are processed in PAIRS with their DVE instruction streams interleaved:
adjacent instructions come from independent groups, hiding the DVE pipe-DRAIN
that back-to-back dependent ops would expose.

Per group: DMA fp16 bones -> DVE crosses(3+3 muls, rot sub)/pc,red/pp(merged,
broadcast operand)/vsums -> ACT relu+square with fp32 accum_out -> PE
ones-matmul reduces coplanarity products into PSUM fp32 -> ACT identity+accum
folds PSUM chunks. Host sums the per-core partials in float64.

pool_units bitmask moves work to GpSimd (1=red, 2=pc, 4=vs) to probe the
DVE/Pool shared-SBUF-port contention question.
"""

import sys

for _p in ("/opt/trn_rl_repo", "/root/.axon_site/_ro/trn_rl_repo"):
    if _p not in sys.path:
        sys.path.append(_p)

import numpy as np

import concourse.bacc as bacc
import concourse.mybir as mybir
from concourse import tile
from concourse.bass_utils import run_bass_kernel_spmd
from contextlib import ExitStack

N_CORES = 8
P = 128          # SBUF partitions
B_FULL = 524288  # total batch
ROW = 60         # 3 comps * 4 bones * 5 fingers per row
DEF_K = 128

F16 = mybir.dt.float16
F32 = mybir.dt.float32


def build_bass_paired(rows_per_core: int, K: int, reps: int = 1,
                      hw_loop: int = 1, sreset: bool = False,
                      xbufs: int = 2, stages: int = 9, inplace_rot: bool = False,
                      dma_tiny: bool = False, pair_dma: bool = False,
                      dma_q: int = 0, pp_split: bool = False,
                      dma_pace: bool = False):
    """v6: the W=2 group pair shares double-width tiles so each DVE
    instruction covers both groups — 22 DVE instructions per pass instead of
    36, amortizing the ~280ns/instruction issue overhead measured on HW.

    Tiles touched only by the DVE (m1/m2/rot/pp) live in single-buffer pools:
    the DVE executes serially, so WAR/WAW hazards against its own later
    instructions cannot overlap anyway. Only tiles crossing engines (xb: DMA,
    red: PE, v/vs: ACT) are double-buffered.
    """
    W = 2
    assert rows_per_core % (P * K * W) == 0
    G = rows_per_core // (P * K)
    NP = G // W                # pairs per pass
    S5 = 5 * K
    CB = 4 * S5
    FK = 3 * CB
    NCOP = 3 * S5
    NR = 3 * NCOP
    NV = 2 * S5
    RED_N = 2 * NCOP           # fused (palm,mid)*b4 products per group

    nc = bacc.Bacc("TRN2", target_bir_lowering=False, debug=False)

    x = nc.dram_tensor("x", [G, P, FK], F16, kind="ExternalInput")
    n_chunks = (RED_N + 511) // 512
    cop_out = nc.dram_tensor("cop_out", [1, n_chunks], F32, kind="ExternalOutput")
    mask_out = nc.dram_tensor("mask_out", [P, NP * reps], F32,
                              kind="ExternalOutput")

    with tile.TileContext(nc) as tc, ExitStack() as ctx:
        xpool = ctx.enter_context(tc.tile_pool(name="xpool", bufs=xbufs))
        mpool = ctx.enter_context(tc.tile_pool(name="mpool", bufs=1))
        rpool = ctx.enter_context(tc.tile_pool(name="rpool", bufs=2))
        vpool = ctx.enter_context(tc.tile_pool(name="vpool", bufs=2))
        spool = ctx.enter_context(tc.tile_pool(name="spool", bufs=1))
        psum = ctx.enter_context(tc.tile_pool(name="psum", bufs=1, space="PSUM"))

        ones = spool.tile([P, 1], F16)
        nc.gpsimd.memset(ones[:], 1.0)
        acc = spool.tile([P, NP * reps], F32)
        psum_cop = [psum.tile([1, 512], F32, name=f"ps{j}", tag=f"ps{j}")
                    for j in range(n_chunks)]
        nc.gpsimd.memset(acc[:], 0.0)
        cop_acc = spool.tile([1, n_chunks], F32)
        nc.gpsimd.memset(cop_acc[:], 0.0)

        loop_cm = (tc.For_i(0, hw_loop, 1, staggered_reset=sreset)
                   if hw_loop > 1 else None)
        if loop_cm is not None:
            loop_cm.__enter__()

        for rep in range(reps):
            for p0 in range(NP):
                g0 = p0 * W
                xb = xpool.tile([P, W * FK], F16, tag="xb", name="xb")
                if dma_tiny:
                    nc.sync.dma_start(xb[:, 0:128], x.ap()[g0][:, 0:128])
                elif pair_dma:
                    nc.sync.dma_start(
                        xb[:].rearrange("p (w f) -> w p f", w=W),
                        x.ap()[g0 : g0 + W])
                else:
                    for i in range(W):
                        qeng = nc.scalar if (dma_q and i % 2) else nc.sync
                        qeng.dma_start(xb[:, i * FK : (i + 1) * FK],
                                       x.ap()[g0 + i])
                if stages < 1:
                    continue

                # ---- cross products over both groups at once ---------------
                m1 = mpool.tile([P, W * NR], F16, tag="m1", name="m1")
                m2 = mpool.tile([P, W * NR], F16, tag="m2", name="m2")
                rot = m1 if inplace_rot else mpool.tile([P, W * NR], F16,
                                                        tag="rot", name="rot")
                xw = xb[:].rearrange("p (w c m) -> p w c m", w=W, c=3)
                for mt, a_cs, b_cs in (
                    (m1, slice(1, 3), slice(2, None, -2)),
                    (m2, slice(2, None, -2), slice(1, 3)),
                ):
                    nc.vector.tensor_mul(
                        mt[:].rearrange("p (w c m) -> p w c m", w=W, c=3)
                        [:, :, 0:2],
                        xw[:, :, a_cs, S5 : S5 + NCOP],
                        xw[:, :, b_cs, 0:NCOP])
                for mt, a_c, b_c in ((m1, 0, 1), (m2, 1, 0)):
                    nc.vector.tensor_mul(
                        mt[:].rearrange("p (w c m) -> p w c m", w=W, c=3)
                        [:, :, 2:3],
                        xw[:, :, a_c : a_c + 1, S5 : S5 + NCOP],
                        xw[:, :, b_c : b_c + 1, 0:NCOP])
                if stages >= 2:
                    nc.vector.tensor_sub(rot[:], m1[:], m2[:])

                last_pair = p0 == NP - 1 and rep == reps - 1

                def emit_cop_path():
                    # red[w][c][q][n] = rot[w][c][q][n]*b4[w][c][n], q in
                    # {palm,mid}; (palm+mid) summing rides the PE ones-reduce
                    red = rpool.tile([P, W * RED_N], F16, tag="red", name="red")
                    for i in range(W):
                        rv = rot[:].rearrange("p (w c q n) -> p w c q n",
                                              w=W, c=3, q=3)[:, i]
                        bv = xb[:].rearrange("p (w c q n) -> p w c q n",
                                             w=W, c=3, q=4)[:, i]
                        nc.vector.tensor_mul(
                            red[:, i * RED_N : (i + 1) * RED_N].rearrange(
                                "p (c q n) -> p c q n", c=3, q=2),
                            rv[:, :, 0:2],
                            bv[:, :, 3:4].broadcast_to([P, 3, 2, S5]))
                    for i in range(W):
                        first = rep == 0 and p0 == 0 and i == 0
                        last = last_pair and i == W - 1
                        for j in range(n_chunks):
                            lo = 512 * j
                            hi = min(RED_N, lo + 512)
                            nc.tensor.matmul(
                                psum_cop[j][:, 0 : hi - lo], ones[:],
                                red[:, i * RED_N + lo : i * RED_N + hi],
                                start=first, stop=last)

                if last_pair and stages >= 5:
                    emit_cop_path()
                if stages < 3:
                    continue

                # ---- v1 = tip.mid, v2 = palm.mid ---------------------------
                pp = mpool.tile([P, W * 6 * S5], F16, tag="pp", name="pp")
                if pp_split:
                    # broadcast-free: one plain strided mul per (group, dot)
                    for i in range(W):
                        rq = rot[:].rearrange("p (w c q n) -> p w c q n",
                                              w=W, c=3, q=3)[:, i]
                        ppv = pp[:, i * 6 * S5 : (i + 1) * 6 * S5].rearrange(
                            "p (c w2 n) -> p c w2 n", c=3, w2=2)
                        for w2, qsel in ((0, 0), (1, 2)):  # v2=palm, v1=tip
                            nc.vector.tensor_mul(
                                ppv[:, :, w2 : w2 + 1],
                                rq[:, :, qsel : qsel + 1],
                                rq[:, :, 1:2])
                else:
                    for i in range(W):
                        rq = rot[:].rearrange("p (w c q n) -> p w q c n",
                                              w=W, c=3, q=3)[:, i]
                        ppv = pp[:, i * 6 * S5 : (i + 1) * 6 * S5].rearrange(
                            "p (c w2 n) -> p w2 c n", c=3, w2=2)
                        nc.vector.tensor_mul(ppv, rq[:, 0:3:2],
                                             rq[:, 1:2].broadcast_to([P, 2, 3, S5]))
                vs = vpool.tile([P, W * NV], F16, tag="vs", name="vs")
                v = vpool.tile([P, W * NV], F16, tag="v", name="v")
                ppw = pp[:].rearrange("p (w d) -> p w d", w=W)
                nc.vector.tensor_add(
                    vs[:].rearrange("p (w d) -> p w d", w=W),
                    ppw[:, :, 0:NV], ppw[:, :, NV : 2 * NV])
                nc.vector.tensor_add(
                    v[:].rearrange("p (w d) -> p w d", w=W),
                    vs[:].rearrange("p (w d) -> p w d", w=W),
                    ppw[:, :, 2 * NV : 3 * NV])

                # ---- masked squares: sum(relu(-v)^2) -> acc ----------------
                if stages >= 4:
                    col = rep * NP + p0
                    nc.scalar.activation(
                        vs[:], v[:],
                        mybir.ActivationFunctionType.Relu, scale=-1.0)
                    nc.scalar.activation(
                        v[:], vs[:],
                        mybir.ActivationFunctionType.Square,
                        accum_out=acc[:, col : col + 1])

                if stages < 5:
                    continue
                if not last_pair:
                    emit_cop_path()

        if loop_cm is not None:
            loop_cm.__exit__(None, None, None)

        # PSUM fold hoisted out of the For_i loop: accumulation restarts each
        # iteration (start=True on the first matmul), so only the final
        # iteration's PSUM contents matter — folding per iteration would just
        # serialize ~5us of ACT work into every loop boundary.
        if stages >= 5:
            ps_scratch = spool.tile([1, 512], F32, name="pss")
            for j in range(n_chunks):
                lo = 512 * j
                hi = min(RED_N, lo + 512)
                nc.scalar.activation(
                    ps_scratch[:, 0 : hi - lo],
                    psum_cop[j][:, 0 : hi - lo],
                    mybir.ActivationFunctionType.Copy,
                    accum_out=cop_acc[:, j : j + 1])

        nc.sync.dma_start(cop_out.ap(), cop_acc[:])
        nc.scalar.dma_start(mask_out.ap(), acc[:])

    nc.compile()
    return nc, G


def build_bass(rows_per_core: int, K: int, reps: int = 1, hw_loop: int = 1,
               pool_units: int = 0, W: int = 2, sreset: bool = False,
               act_relu: bool = True, xbufs: int = 0, vbufs: int = 0,
               fuse_red: bool = True, merge_m12: bool = True,
               stages: int = 9, paired: int = 1, inplace_rot: int = 0,
               dma_tiny: int = 0, pair_dma: int = 0, dma_q: int = 0,
               pp_split: int = 0, dma_pace: int = 0):
    if paired:
        return build_bass_paired(rows_per_core, K, reps=reps, hw_loop=hw_loop,
                                 sreset=bool(sreset), xbufs=xbufs or 2,
                                 stages=stages, inplace_rot=bool(inplace_rot),
                                 dma_tiny=bool(dma_tiny), pair_dma=bool(pair_dma),
                                 dma_q=dma_q, pp_split=bool(pp_split),
                                 dma_pace=bool(dma_pace))
    return build_bass_v5(rows_per_core, K, reps=reps, hw_loop=hw_loop,
                         pool_units=pool_units, W=W, sreset=sreset,
                         act_relu=act_relu, xbufs=xbufs, vbufs=vbufs,
                         fuse_red=fuse_red, merge_m12=merge_m12, stages=stages)


def build_bass_v5(rows_per_core: int, K: int, reps: int = 1, hw_loop: int = 1,
                  pool_units: int = 0, W: int = 2, sreset: bool = False,
                  act_relu: bool = True, xbufs: int = 0, vbufs: int = 0,
                  fuse_red: bool = True, merge_m12: bool = True,
                  stages: int = 9):
    """rows_per_core = P * K * G.  K = rows per partition slot per group.

    reps>1 unrolls the compute (timing); hw_loop>1 wraps it in a device-side
    For_i (timing; outputs = last iteration's = one correct pass).
    pool_units bitmask in {1:red, 2:pc, 4:vs} moves that op to GpSimd.
    act_relu False puts relu on DVE tensor_scalar (4x) instead of ACT.
    xbufs/vbufs override pool depths (default 2W / W+1) for pipelining.
    fuse_red folds pc+red into one broadcast multiply over [palm,mid].
    """
    assert rows_per_core % (P * K * W) == 0
    G = rows_per_core // (P * K)
    S5 = 5 * K            # one [f][k] slab
    CB = 4 * S5           # bone elems per component [q][f][k]
    FK = 3 * CB           # fp16 elems per partition per group (60*K)
    NCOP = 3 * S5         # coplane products per partition (also m1/m2 per c)
    NR = 3 * NCOP         # rot elems per partition
    NV = 2 * S5           # v values per partition ({v2,v1} x [f][k])

    nc = bacc.Bacc("TRN2", target_bir_lowering=False, debug=False)

    x = nc.dram_tensor("x", [G, P, FK], F16, kind="ExternalInput")
    RED_N = 2 * NCOP if fuse_red else NCOP
    n_chunks = (RED_N + 511) // 512
    cop_out = nc.dram_tensor("cop_out", [1, n_chunks], F32, kind="ExternalOutput")
    mask_out = nc.dram_tensor("mask_out", [P, G * reps], F32, kind="ExternalOutput")

    if not xbufs:
        xbufs = 2 * W
    if not vbufs:
        vbufs = W + 1

    with tile.TileContext(nc) as tc, ExitStack() as ctx:
        xpool = ctx.enter_context(tc.tile_pool(name="xpool", bufs=xbufs))
        mpool = ctx.enter_context(tc.tile_pool(name="mpool", bufs=W))
        vpool = ctx.enter_context(tc.tile_pool(name="vpool", bufs=vbufs))
        spool = ctx.enter_context(tc.tile_pool(name="spool", bufs=1))
        psum = ctx.enter_context(tc.tile_pool(name="psum", bufs=1, space="PSUM"))

        ones = spool.tile([P, 1], F16)
        nc.gpsimd.memset(ones[:], 1.0)
        acc = spool.tile([P, G * reps], F32)
        psum_cop = [psum.tile([1, 512], F32, name=f"ps{j}", tag=f"ps{j}")
                    for j in range(n_chunks)]
        nc.gpsimd.memset(acc[:], 0.0)
        cop_acc = spool.tile([1, n_chunks], F32)
        nc.gpsimd.memset(cop_acc[:], 0.0)

        c3 = lambda ap: ap.rearrange("p (c n) -> p c n", c=3)

        loop_cm = (tc.For_i(0, hw_loop, 1, staggered_reset=sreset)
                   if hw_loop > 1 else None)
        if loop_cm is not None:
            loop_cm.__enter__()

        for rep in range(reps):
            for g0 in range(0, G, W):
                pair = tuple(range(g0, g0 + W))
                st = [{} for _ in range(W)]  # per-group tile state

                for i, g in enumerate(pair):
                    xb = xpool.tile([P, FK], F16, tag="xb", name="xb")
                    nc.sync.dma_start(xb[:], x.ap()[g])
                    st[i]["xb"] = xb

                if stages < 1:
                    continue
                # ---- cross products, c-major [c][q:palm,mid,tip][f][k] ------
                # rot[c][q] = B_{c1}[q+1]*B_{c2}[q] - B_{c2}[q+1]*B_{c1}[q]
                for i in range(W):
                    st[i]["m1"] = mpool.tile([P, NR], F16, tag="m1", name="m1")
                    st[i]["m2"] = mpool.tile([P, NR], F16, tag="m2", name="m2")
                    st[i]["rot"] = mpool.tile([P, NR], F16, tag="rot", name="rot")
                if merge_m12:
                    # c in {0,1} fused per m-tensor: operand c-strides are
                    # affine there (+CB / -2CB); c=2 wraps, emitted alone
                    for which, a_cs, b_cs in (
                        ("m1", slice(1, 3), slice(2, None, -2)),
                        ("m2", slice(2, None, -2), slice(1, 3)),
                    ):
                        for i in range(W):
                            xv = st[i]["xb"][:].rearrange(
                                "p (c m) -> p c m", c=3)
                            nc.vector.tensor_mul(
                                st[i][which][:, 0 : 2 * NCOP].rearrange(
                                    "p (c m) -> p c m", c=2),
                                xv[:, a_cs, S5 : S5 + NCOP],
                                xv[:, b_cs, 0:NCOP])
                    for which, a_c, b_c in (("m1", 0, 1), ("m2", 1, 0)):
                        for i in range(W):
                            xv = st[i]["xb"][:].rearrange(
                                "p (c m) -> p c m", c=3)
                            nc.vector.tensor_mul(
                                st[i][which][:, 2 * NCOP : 3 * NCOP],
                                xv[:, a_c, S5 : S5 + NCOP],
                                xv[:, b_c, 0:NCOP])
                else:
                    for c in range(3):
                        c1, c2 = (c + 1) % 3, (c + 2) % 3
                        for which, a_off, b_off in (
                            ("m1", c1 * CB + S5, c2 * CB),
                            ("m2", c2 * CB + S5, c1 * CB),
                        ):
                            for i in range(W):
                                xb = st[i]["xb"]
                                nc.vector.tensor_mul(
                                    st[i][which][:, c * NCOP : (c + 1) * NCOP],
                                    xb[:, a_off : a_off + NCOP],
                                    xb[:, b_off : b_off + NCOP])
                if stages >= 2:
                    for i in range(W):
                        nc.vector.tensor_sub(st[i]["rot"][:], st[i]["m1"][:],
                                             st[i]["m2"][:])

                last_pair = g0 + W >= G and rep == reps - 1

                def emit_cop_path():
                    # ---- coplane products -----------------------------------
                    red_eng = nc.gpsimd if (pool_units & 1) else nc.vector
                    if fuse_red:
                        # red2[c][q][n] = rot[c][q][n] * b4[c][n], q in
                        # {palm, mid}; the (palm+mid) add is deferred to the
                        # PE ones-reduce (linear), saving one DVE op
                        for i in range(W):
                            st[i]["red"] = vpool.tile([P, RED_N], F16,
                                                      tag="red", name="red")
                        for i in range(W):
                            rv = st[i]["rot"][:].rearrange(
                                "p (c q n) -> p c q n", c=3, q=3)
                            bv = st[i]["xb"][:].rearrange(
                                "p (c q n) -> p c q n", c=3, q=4)
                            red_eng.tensor_mul(
                                st[i]["red"][:].rearrange(
                                    "p (c q n) -> p c q n", c=3, q=2),
                                rv[:, :, 0:2],
                                bv[:, :, 3:4].broadcast_to([P, 3, 2, S5]))
                    else:
                        # pc = palm + mid; red = pc * b4
                        for i in range(W):
                            st[i]["pc"] = vpool.tile([P, NCOP], F16, tag="pc",
                                                     name="pc")
                            st[i]["red"] = vpool.tile([P, NCOP], F16,
                                                      tag="red", name="red")
                        pc_eng = nc.gpsimd if (pool_units & 2) else nc.vector
                        for i in range(W):
                            rv = c3(st[i]["rot"][:])
                            pc_eng.tensor_add(c3(st[i]["pc"][:]),
                                              rv[:, :, 0:S5],
                                              rv[:, :, S5 : 2 * S5])
                        for i in range(W):
                            bv = c3(st[i]["xb"][:])
                            red_eng.tensor_mul(c3(st[i]["red"][:]),
                                               c3(st[i]["pc"][:]),
                                               bv[:, :, 3 * S5 : 4 * S5])
                    # ---- PE reduction of coplane products over partitions ---
                    for i, g in enumerate(pair):
                        first = rep == 0 and g == 0
                        last = rep == reps - 1 and g == G - 1
                        for j in range(n_chunks):
                            lo = 512 * j
                            hi = min(RED_N, lo + 512)
                            nc.tensor.matmul(psum_cop[j][:, 0 : hi - lo],
                                             ones[:],
                                             st[i]["red"][:, lo:hi],
                                             start=first, stop=last)

                if last_pair and stages >= 5:
                    emit_cop_path()

                if stages < 3:
                    continue
                # ---- v1 = tip.mid, v2 = palm.mid; pp[c] = [v2_c | v1_c] ----
                for i in range(W):
                    st[i]["pp"] = vpool.tile([P, 6 * S5], F16, tag="pp", name="pp")
                for i in range(W):
                    rq = st[i]["rot"][:].rearrange("p (c q n) -> p q c n", c=3, q=3)
                    ppv = st[i]["pp"][:].rearrange("p (c w n) -> p w c n", c=3, w=2)
                    nc.vector.tensor_mul(ppv, rq[:, 0:3:2],
                                         rq[:, 1:2].broadcast_to([P, 2, 3, S5]))
                for i in range(W):
                    st[i]["vs"] = vpool.tile([P, NV], F16, tag="vs", name="vs")
                    st[i]["v"] = vpool.tile([P, NV], F16, tag="v", name="v")
                vs_eng = nc.gpsimd if (pool_units & 4) else nc.vector
                for i in range(W):
                    pp = st[i]["pp"]
                    vs_eng.tensor_add(st[i]["vs"][:], pp[:, 0:NV],
                                      pp[:, NV : 2 * NV])
                for i in range(W):
                    nc.vector.tensor_add(st[i]["v"][:], st[i]["vs"][:],
                                         st[i]["pp"][:, 2 * NV : 3 * NV])

                # ---- masked squares: sum(relu(-v)^2) -> acc -----------------
                if stages < 4:
                    continue
                for i, g in enumerate(pair):
                    if act_relu:
                        nc.scalar.activation(
                            st[i]["vs"][:], st[i]["v"][:],
                            mybir.ActivationFunctionType.Relu, scale=-1.0)
                    else:
                        nc.vector.tensor_scalar(
                            st[i]["vs"][:], st[i]["v"][:], -1.0, 0.0,
                            mybir.AluOpType.mult, mybir.AluOpType.max)
                    nc.scalar.activation(st[i]["v"][:], st[i]["vs"][:],
                                         mybir.ActivationFunctionType.Square,
                                         accum_out=acc[:, rep * G + g : rep * G + g + 1])

                if stages < 5:
                    continue
                if not last_pair:
                    emit_cop_path()
                else:
                    # fold PSUM chunks on ACT (identity + accum); ACT reads
                    # PSUM cheaply and is otherwise idle at the tail
                    ps_scratch = vpool.tile([1, 512], F32, tag="pss", name="pss")
                    for j in range(n_chunks):
                        lo = 512 * j
                        hi = min(RED_N, lo + 512)
                        nc.scalar.activation(
                            ps_scratch[:, 0 : hi - lo],
                            psum_cop[j][:, 0 : hi - lo],
                            mybir.ActivationFunctionType.Copy,
                            accum_out=cop_acc[:, j : j + 1])

        if loop_cm is not None:
            loop_cm.__exit__(None, None, None)

        # ---- epilogue: DMA out --------------------------------------------
        nc.sync.dma_start(cop_out.ap(), cop_acc[:])
        nc.scalar.dma_start(mask_out.ap(), acc[:])

    nc.compile()
    return nc, G


def host_planarize(x: np.ndarray, n_cores: int, K: int) -> np.ndarray:
    """[B,21,3] f32 -> [cores, G, P, 60K] f16: bone layout [c][q:4][f:5][k].

    Bones are differenced in fp32 on host, then cast once to fp16.
    """
    B = x.shape[0]
    R = B // n_cores
    G = R // (P * K)
    xr = x.reshape(n_cores, G, P, K, 21, 3)
    jidx = (np.arange(5) * 4)[:, None] + np.arange(5)[None, :]  # [f, jj]
    xj = xr[:, :, :, :, jidx, :]                 # [cores,G,P,K,f,jj,3]
    bones = xj[:, :, :, :, :, 1:, :] - xj[:, :, :, :, :, :-1, :]
    xp = bones.transpose(0, 1, 2, 6, 5, 4, 3)    # [cores,G,P,c,q,f,K]
    out = np.empty((n_cores, G, P, ROW * K), dtype=np.float16)
    np.copyto(out.reshape(xp.shape), xp)
    return out


_CACHE = {}


def _get_nc(rows_per_core: int, K: int):
    key = (rows_per_core, K)
    if key not in _CACHE:
        _CACHE[key] = build_bass(rows_per_core, K)
    return _CACHE[key]


def kernel(pose23d_pred: np.ndarray) -> np.ndarray:
    x = np.asarray(pose23d_pred, dtype=np.float32)
    assert x.shape == (B_FULL, 21, 3), x.shape
    K = DEF_K
    R = B_FULL // N_CORES
    nc, G = _get_nc(R, K)
    xp = host_planarize(x, N_CORES, K)
    in_maps = [{"x": xp[i]} for i in range(N_CORES)]
    res = run_bass_kernel_spmd(nc, in_maps, list(range(N_CORES)))
    total = 0.0
    for r in res.results:
        total += r["cop_out"].astype(np.float64).sum()
        total += r["mask_out"].astype(np.float64).sum()
    return np.float32(total)


# revision 20
# speedup vs baseline: 1.0047x; 1.0047x over previous
"""JointAngleLoss Trainium2 kernel (8-core data-parallel), v6.

Input : pose23d_pred [524288, 21, 3] float32
Output: scalar float32 loss (matches reference.reference)

Strategy: pure data-parallel over the batch dim; each of 8 NeuronCores handles
65536 rows. Host pre-permutes the input into a per-partition bone layout
BONES[c][q][f][k] (bones differenced on host in fp32, then cast once to fp16;
end-to-end loss error ~7e-6 vs the 2e-2 gate), so every device-side vector
operand is a contiguous fp16 slice (DVE 2x_1P packed mode). Shipping bones
(60 values/row) instead of joints (75) cuts both DMA bytes and the on-device
bone-subtraction stage.

The two groups of each scheduling pair share double-width tiles, so one DVE
instruction covers both groups (22 TT instructions per pass). Tiles touched
only by the DVE (m1/m2/rot/pp) are single-buffered — the engine is serial, so
its own WAR/WAW hazards cannot overlap; only cross-engine tiles (xb: DMA-
written, red: PE-read, v/vs: ACT-read) are multi-buffered, with the DMA input
pool two pairs deep so transfers prefetch under compute.

Per pair: DMA fp16 bones -> DVE crosses (m1/m2 muls with c0,c1 fused via
affine/negative-stride APs, rot sub) -> DVE pp (palm,tip x mid, broadcast
operand) / v-sums -> ACT relu + square with fp32 accum_out -> DVE red
(rot[palm,mid]*b4 broadcast; the palm+mid add rides the linear PE reduce) ->
PE ones-matmul reduces coplanarity products into PSUM fp32. The PSUM fold
(ACT Copy+accum, which sits near PSUM) is hoisted out of the timing For_i
loop. Host sums the per-core partials in float64.

Measured (this container): 23.5us/pass DMA floor, ~59us DVE busy (cost
model), ~64us sustained per pass on HW. GpSimd offload and fp8 were measured
or reasoned strictly worse (Pool shares a DVE SBUF port at 0.42 efficiency;
fp8 loses the DVE 2x packed mode).
"""

import sys

for _p in ("/opt/trn_rl_repo", "/root/.axon_site/_ro/trn_rl_repo"):
    if _p not in sys.path:
        sys.path.append(_p)

import numpy as np

import concourse.bacc as bacc
import concourse.mybir as mybir
from concourse import tile
from concourse.bass_utils import run_bass_kernel_spmd
from contextlib import ExitStack

N_CORES = 8
P = 128          # SBUF partitions
B_FULL = 524288  # total batch
ROW = 60         # 3 comps * 4 bones * 5 fingers per row
DEF_K = 128

F16 = mybir.dt.float16
F32 = mybir.dt.float32


def build_bass_paired(rows_per_core: int, K: int, reps: int = 1,
                      hw_loop: int = 1, sreset: bool = False,
                      xbufs: int = 2, stages: int = 9, inplace_rot: bool = False,
                      dma_tiny: bool = False, pair_dma: bool = False,
                      dma_q: int = 0, pp_split: bool = False,
                      dma_pace: bool = False):
    """v6: the W=2 group pair shares double-width tiles so each DVE
    instruction covers both groups — 22 DVE instructions per pass instead of
    36, amortizing the ~280ns/instruction issue overhead measured on HW.

    Tiles touched only by the DVE (m1/m2/rot/pp) live in single-buffer pools:
    the DVE executes serially, so WAR/WAW hazards against its own later
    instructions cannot overlap anyway. Only tiles crossing engines (xb: DMA,
    red: PE, v/vs: ACT) are double-buffered.
    """
    W = 2
    assert rows_per_core % (P * K * W) == 0
    G = rows_per_core // (P * K)
    NP = G // W                # pairs per pass
    S5 = 5 * K
    CB = 4 * S5
    FK = 3 * CB
    NCOP = 3 * S5
    NR = 3 * NCOP
    NV = 2 * S5
    RED_N = 2 * NCOP           # fused (palm,mid)*b4 products per group

    nc = bacc.Bacc("TRN2", target_bir_lowering=False, debug=False)

    x = nc.dram_tensor("x", [G, P, FK], F16, kind="ExternalInput")
    n_chunks = (RED_N + 511) // 512
    cop_out = nc.dram_tensor("cop_out", [1, n_chunks], F32, kind="ExternalOutput")
    mask_out = nc.dram_tensor("mask_out", [P, NP * reps], F32,
                              kind="ExternalOutput")

    with tile.TileContext(nc) as tc, ExitStack() as ctx:
        xpool = ctx.enter_context(tc.tile_pool(name="xpool", bufs=xbufs))
        mpool = ctx.enter_context(tc.tile_pool(name="mpool", bufs=1))
        rpool = ctx.enter_context(tc.tile_pool(name="rpool", bufs=2))
        vpool = ctx.enter_context(tc.tile_pool(name="vpool", bufs=2))
        spool = ctx.enter_context(tc.tile_pool(name="spool", bufs=1))
        psum = ctx.enter_context(tc.tile_pool(name="psum", bufs=1, space="PSUM"))

        ones = spool.tile([P, 1], F16)
        nc.gpsimd.memset(ones[:], 1.0)
        acc = spool.tile([P, NP * reps], F32)
        psum_cop = [psum.tile([1, 512], F32, name=f"ps{j}", tag=f"ps{j}")
                    for j in range(n_chunks)]
        nc.gpsimd.memset(acc[:], 0.0)
        cop_acc = spool.tile([1, n_chunks], F32)
        nc.gpsimd.memset(cop_acc[:], 0.0)

        loop_cm = (tc.For_i(0, hw_loop, 1, staggered_reset=sreset)
                   if hw_loop > 1 else None)
        if loop_cm is not None:
            loop_cm.__enter__()

        for rep in range(reps):
            for p0 in range(NP):
                g0 = p0 * W
                xb = xpool.tile([P, W * FK], F16, tag="xb", name="xb")
                if dma_tiny:
                    nc.sync.dma_start(xb[:, 0:128], x.ap()[g0][:, 0:128])
                elif pair_dma:
                    nc.sync.dma_start(
                        xb[:].rearrange("p (w f) -> w p f", w=W),
                        x.ap()[g0 : g0 + W])
                else:
                    for i in range(W):
                        qeng = nc.scalar if (dma_q and i % 2) else nc.sync
                        qeng.dma_start(xb[:, i * FK : (i + 1) * FK],
                                       x.ap()[g0 + i])
                if stages < 1:
                    continue

                # ---- cross products over both groups at once ---------------
                m1 = mpool.tile([P, W * NR], F16, tag="m1", name="m1")
                m2 = mpool.tile([P, W * NR], F16, tag="m2", name="m2")
                rot = m1 if inplace_rot else mpool.tile([P, W * NR], F16,
                                                        tag="rot", name="rot")
                xw = xb[:].rearrange("p (w c m) -> p w c m", w=W, c=3)
                for mt, a_cs, b_cs in (
                    (m1, slice(1, 3), slice(2, None, -2)),
                    (m2, slice(2, None, -2), slice(1, 3)),
                ):
                    nc.vector.tensor_mul(
                        mt[:].rearrange("p (w c m) -> p w c m", w=W, c=3)
                        [:, :, 0:2],
                        xw[:, :, a_cs, S5 : S5 + NCOP],
                        xw[:, :, b_cs, 0:NCOP])
                for mt, a_c, b_c in ((m1, 0, 1), (m2, 1, 0)):
                    nc.vector.tensor_mul(
                        mt[:].rearrange("p (w c m) -> p w c m", w=W, c=3)
                        [:, :, 2:3],
                        xw[:, :, a_c : a_c + 1, S5 : S5 + NCOP],
                        xw[:, :, b_c : b_c + 1, 0:NCOP])
                if stages >= 2:
                    nc.vector.tensor_sub(rot[:], m1[:], m2[:])

                last_pair = p0 == NP - 1 and rep == reps - 1

                def emit_cop_path():
                    # red[w][c][q][n] = rot[w][c][q][n]*b4[w][c][n], q in
                    # {palm,mid}; (palm+mid) summing rides the PE ones-reduce
                    red = rpool.tile([P, W * RED_N], F16, tag="red", name="red")
                    for i in range(W):
                        rv = rot[:].rearrange("p (w c q n) -> p w c q n",
                                              w=W, c=3, q=3)[:, i]
                        bv = xb[:].rearrange("p (w c q n) -> p w c q n",
                                             w=W, c=3, q=4)[:, i]
                        nc.vector.tensor_mul(
                            red[:, i * RED_N : (i + 1) * RED_N].rearrange(
                                "p (c q n) -> p c q n", c=3, q=2),
                            rv[:, :, 0:2],
                            bv[:, :, 3:4].broadcast_to([P, 3, 2, S5]))
                    for i in range(W):
                        first = rep == 0 and p0 == 0 and i == 0
                        last = last_pair and i == W - 1
                        for j in range(n_chunks):
                            lo = 512 * j
                            hi = min(RED_N, lo + 512)
                            nc.tensor.matmul(
                                psum_cop[j][:, 0 : hi - lo], ones[:],
                                red[:, i * RED_N + lo : i * RED_N + hi],
                                start=first, stop=last)

                if last_pair and stages >= 5:
                    emit_cop_path()
                if stages < 3:
                    continue

                # ---- v1 = tip.mid, v2 = palm.mid ---------------------------
                pp = mpool.tile([P, W * 6 * S5], F16, tag="pp", name="pp")
                if pp_split:
                    # broadcast-free: one plain strided mul per (group, dot)
                    for i in range(W):
                        rq = rot[:].rearrange("p (w c q n) -> p w c q n",
                                              w=W, c=3, q=3)[:, i]
                        ppv = pp[:, i * 6 * S5 : (i + 1) * 6 * S5].rearrange(
                            "p (c w2 n) -> p c w2 n", c=3, w2=2)
                        for w2, qsel in ((0, 0), (1, 2)):  # v2=palm, v1=tip
                            nc.vector.tensor_mul(
                                ppv[:, :, w2 : w2 + 1],
                                rq[:, :, qsel : qsel + 1],
                                rq[:, :, 1:2])
                else:
                    for i in range(W):
                        rq = rot[:].rearrange("p (w c q n) -> p w q c n",
                                              w=W, c=3, q=3)[:, i]
                        ppv = pp[:, i * 6 * S5 : (i + 1) * 6 * S5].rearrange(
                            "p (c w2 n) -> p w2 c n", c=3, w2=2)
                        nc.vector.tensor_mul(ppv, rq[:, 0:3:2],
                                             rq[:, 1:2].broadcast_to([P, 2, 3, S5]))
                vs = vpool.tile([P, W * NV], F16, tag="vs", name="vs")
                v = vpool.tile([P, W * NV], F16, tag="v", name="v")
                ppw = pp[:].rearrange("p (w d) -> p w d", w=W)
                nc.vector.tensor_add(
                    vs[:].rearrange("p (w d) -> p w d", w=W),
                    ppw[:, :, 0:NV], ppw[:, :, NV : 2 * NV])
                nc.vector.tensor_add(
                    v[:].rearrange("p (w d) -> p w d", w=W),
                    vs[:].rearrange("p (w d) -> p w d", w=W),
                    ppw[:, :, 2 * NV : 3 * NV])

                # ---- masked squares: sum(relu(-v)^2) -> acc ----------------
                if stages >= 4:
                    col = rep * NP + p0
                    nc.scalar.activation(
                        vs[:], v[:],
                        mybir.ActivationFunctionType.Relu, scale=-1.0)
                    nc.scalar.activation(
                        v[:], vs[:],
                        mybir.ActivationFunctionType.Square,
                        accum_out=acc[:, col : col + 1])

                if stages < 5:
                    continue
                if not last_pair:
                    emit_cop_path()

        if loop_cm is not None:
            loop_cm.__exit__(None, None, None)

        # PSUM fold hoisted out of the For_i loop: accumulation restarts each
        # iteration (start=True on the first matmul), so only the final
        # iteration's PSUM contents matter — folding per iteration would just
        # serialize ~5us of ACT work into every loop boundary.
        if stages >= 5:
            ps_scratch = spool.tile([1, 512], F32, name="pss")
            for j in range(n_chunks):
                lo = 512 * j
                hi = min(RED_N, lo + 512)
                nc.scalar.activation(
                    ps_scratch[:, 0 : hi - lo],
                    psum_cop[j][:, 0 : hi - lo],
                    mybir.ActivationFunctionType.Copy,
                    accum_out=cop_acc[:, j : j + 1])

        nc.sync.dma_start(cop_out.ap(), cop_acc[:])
        nc.scalar.dma_start(mask_out.ap(), acc[:])

    nc.compile()
    return nc, G


def build_bass(rows_per_core: int, K: int, reps: int = 1, hw_loop: int = 1,
               pool_units: int = 0, W: int = 2, sreset: bool = False,
               act_relu: bool = True, xbufs: int = 0, vbufs: int = 0,
               fuse_red: bool = True, merge_m12: bool = True,
               stages: int = 9, paired: int = 1, inplace_rot: int = 0,
               dma_tiny: int = 0, pair_dma: int = 0, dma_q: int = 0,
               pp_split: int = 0, dma_pace: int = 0):
    if paired:
        return build_bass_paired(rows_per_core, K, reps=reps, hw_loop=hw_loop,
                                 sreset=bool(sreset), xbufs=xbufs or 2,
                                 stages=stages, inplace_rot=bool(inplace_rot),
                                 dma_tiny=bool(dma_tiny), pair_dma=bool(pair_dma),
                                 dma_q=dma_q, pp_split=bool(pp_split),
                                 dma_pace=bool(dma_pace))
    return build_bass_v5(rows_per_core, K, reps=reps, hw_loop=hw_loop,
                         pool_units=pool_units, W=W, sreset=sreset,
                         act_relu=act_relu, xbufs=xbufs, vbufs=vbufs,
                         fuse_red=fuse_red, merge_m12=merge_m12, stages=stages)


def build_bass_v5(rows_per_core: int, K: int, reps: int = 1, hw_loop: int = 1,
                  pool_units: int = 0, W: int = 2, sreset: bool = False,
                  act_relu: bool = True, xbufs: int = 0, vbufs: int = 0,
                  fuse_red: bool = True, merge_m12: bool = True,
                  stages: int = 9):
    """rows_per_core = P * K * G.  K = rows per partition slot per group.

    reps>1 unrolls the compute (timing); hw_loop>1 wraps it in a device-side
    For_i (timing; outputs = last iteration's = one correct pass).
    pool_units bitmask in {1:red, 2:pc, 4:vs} moves that op to GpSimd.
    act_relu False puts relu on DVE tensor_scalar (4x) instead of ACT.
    xbufs/vbufs override pool depths (default 2W / W+1) for pipelining.
    fuse_red folds pc+red into one broadcast multiply over [palm,mid].
    """
    assert rows_per_core % (P * K * W) == 0
    G = rows_per_core // (P * K)
    S5 = 5 * K            # one [f][k] slab
    CB = 4 * S5           # bone elems per component [q][f][k]
    FK = 3 * CB           # fp16 elems per partition per group (60*K)
    NCOP = 3 * S5         # coplane products per partition (also m1/m2 per c)
    NR = 3 * NCOP         # rot elems per partition
    NV = 2 * S5           # v values per partition ({v2,v1} x [f][k])

    nc = bacc.Bacc("TRN2", target_bir_lowering=False, debug=False)

    x = nc.dram_tensor("x", [G, P, FK], F16, kind="ExternalInput")
    RED_N = 2 * NCOP if fuse_red else NCOP
    n_chunks = (RED_N + 511) // 512
    cop_out = nc.dram_tensor("cop_out", [1, n_chunks], F32, kind="ExternalOutput")
    mask_out = nc.dram_tensor("mask_out", [P, G * reps], F32, kind="ExternalOutput")

    if not xbufs:
        xbufs = 2 * W
    if not vbufs:
        vbufs = W + 1

    with tile.TileContext(nc) as tc, ExitStack() as ctx:
        xpool = ctx.enter_context(tc.tile_pool(name="xpool", bufs=xbufs))
        mpool = ctx.enter_context(tc.tile_pool(name="mpool", bufs=W))
        vpool = ctx.enter_context(tc.tile_pool(name="vpool", bufs=vbufs))
        spool = ctx.enter_context(tc.tile_pool(name="spool", bufs=1))
        psum = ctx.enter_context(tc.tile_pool(name="psum", bufs=1, space="PSUM"))

        ones = spool.tile([P, 1], F16)
        nc.gpsimd.memset(ones[:], 1.0)
        acc = spool.tile([P, G * reps], F32)
        psum_cop = [psum.tile([1, 512], F32, name=f"ps{j}", tag=f"ps{j}")
                    for j in range(n_chunks)]
        nc.gpsimd.memset(acc[:], 0.0)
        cop_acc = spool.tile([1, n_chunks], F32)
        nc.gpsimd.memset(cop_acc[:], 0.0)

        c3 = lambda ap: ap.rearrange("p (c n) -> p c n", c=3)

        loop_cm = (tc.For_i(0, hw_loop, 1, staggered_reset=sreset)
                   if hw_loop > 1 else None)
        if loop_cm is not None:
            loop_cm.__enter__()

        for rep in range(reps):
            for g0 in range(0, G, W):
                pair = tuple(range(g0, g0 + W))
                st = [{} for _ in range(W)]  # per-group tile state

                for i, g in enumerate(pair):
                    xb = xpool.tile([P, FK], F16, tag="xb", name="xb")
                    nc.sync.dma_start(xb[:], x.ap()[g])
                    st[i]["xb"] = xb

                if stages < 1:
                    continue
                # ---- cross products, c-major [c][q:palm,mid,tip][f][k] ------
                # rot[c][q] = B_{c1}[q+1]*B_{c2}[q] - B_{c2}[q+1]*B_{c1}[q]
                for i in range(W):
                    st[i]["m1"] = mpool.tile([P, NR], F16, tag="m1", name="m1")
                    st[i]["m2"] = mpool.tile([P, NR], F16, tag="m2", name="m2")
                    st[i]["rot"] = mpool.tile([P, NR], F16, tag="rot", name="rot")
                if merge_m12:
                    # c in {0,1} fused per m-tensor: operand c-strides are
                    # affine there (+CB / -2CB); c=2 wraps, emitted alone
                    for which, a_cs, b_cs in (
                        ("m1", slice(1, 3), slice(2, None, -2)),
                        ("m2", slice(2, None, -2), slice(1, 3)),
                    ):
                        for i in range(W):
                            xv = st[i]["xb"][:].rearrange(
                                "p (c m) -> p c m", c=3)
                            nc.vector.tensor_mul(
                                st[i][which][:, 0 : 2 * NCOP].rearrange(
                                    "p (c m) -> p c m", c=2),
                                xv[:, a_cs, S5 : S5 + NCOP],
                                xv[:, b_cs, 0:NCOP])
                    for which, a_c, b_c in (("m1", 0, 1), ("m2", 1, 0)):
                        for i in range(W):
                            xv = st[i]["xb"][:].rearrange(
                                "p (c m) -> p c m", c=3)
                            nc.vector.tensor_mul(
                                st[i][which][:, 2 * NCOP : 3 * NCOP],
                                xv[:, a_c, S5 : S5 + NCOP],
                                xv[:, b_c, 0:NCOP])
                else:
                    for c in range(3):
                        c1, c2 = (c + 1) % 3, (c + 2) % 3
                        for which, a_off, b_off in (
                            ("m1", c1 * CB + S5, c2 * CB),
                            ("m2", c2 * CB + S5, c1 * CB),
                        ):
                            for i in range(W):
                                xb = st[i]["xb"]
                                nc.vector.tensor_mul(
                                    st[i][which][:, c * NCOP : (c + 1) * NCOP],
                                    xb[:, a_off : a_off + NCOP],
                                    xb[:, b_off : b_off + NCOP])
                if stages >= 2:
                    for i in range(W):
                        nc.vector.tensor_sub(st[i]["rot"][:], st[i]["m1"][:],
                                             st[i]["m2"][:])

                last_pair = g0 + W >= G and rep == reps - 1

                def emit_cop_path():
                    # ---- coplane products -----------------------------------
                    red_eng = nc.gpsimd if (pool_units & 1) else nc.vector
                    if fuse_red:
                        # red2[c][q][n] = rot[c][q][n] * b4[c][n], q in
                        # {palm, mid}; the (palm+mid) add is deferred to the
                        # PE ones-reduce (linear), saving one DVE op
                        for i in range(W):
                            st[i]["red"] = vpool.tile([P, RED_N], F16,
                                                      tag="red", name="red")
                        for i in range(W):
                            rv = st[i]["rot"][:].rearrange(
                                "p (c q n) -> p c q n", c=3, q=3)
                            bv = st[i]["xb"][:].rearrange(
                                "p (c q n) -> p c q n", c=3, q=4)
                            red_eng.tensor_mul(
                                st[i]["red"][:].rearrange(
                                    "p (c q n) -> p c q n", c=3, q=2),
                                rv[:, :, 0:2],
                                bv[:, :, 3:4].broadcast_to([P, 3, 2, S5]))
                    else:
                        # pc = palm + mid; red = pc * b4
                        for i in range(W):
                            st[i]["pc"] = vpool.tile([P, NCOP], F16, tag="pc",
                                                     name="pc")
                            st[i]["red"] = vpool.tile([P, NCOP], F16,
                                                      tag="red", name="red")
                        pc_eng = nc.gpsimd if (pool_units & 2) else nc.vector
                        for i in range(W):
                            rv = c3(st[i]["rot"][:])
                            pc_eng.tensor_add(c3(st[i]["pc"][:]),
                                              rv[:, :, 0:S5],
                                              rv[:, :, S5 : 2 * S5])
                        for i in range(W):
                            bv = c3(st[i]["xb"][:])
                            red_eng.tensor_mul(c3(st[i]["red"][:]),
                                               c3(st[i]["pc"][:]),
                                               bv[:, :, 3 * S5 : 4 * S5])
                    # ---- PE reduction of coplane products over partitions ---
                    for i, g in enumerate(pair):
                        first = rep == 0 and g == 0
                        last = rep == reps - 1 and g == G - 1
                        for j in range(n_chunks):
                            lo = 512 * j
                            hi = min(RED_N, lo + 512)
                            nc.tensor.matmul(psum_cop[j][:, 0 : hi - lo],
                                             ones[:],
                                             st[i]["red"][:, lo:hi],
                                             start=first, stop=last)

                if last_pair and stages >= 5:
                    emit_cop_path()

                if stages < 3:
                    continue
                # ---- v1 = tip.mid, v2 = palm.mid; pp[c] = [v2_c | v1_c] ----
                for i in range(W):
                    st[i]["pp"] = vpool.tile([P, 6 * S5], F16, tag="pp", name="pp")
                for i in range(W):
                    rq = st[i]["rot"][:].rearrange("p (c q n) -> p q c n", c=3, q=3)
                    ppv = st[i]["pp"][:].rearrange("p (c w n) -> p w c n", c=3, w=2)
                    nc.vector.tensor_mul(ppv, rq[:, 0:3:2],
                                         rq[:, 1:2].broadcast_to([P, 2, 3, S5]))
                for i in range(W):
                    st[i]["vs"] = vpool.tile([P, NV], F16, tag="vs", name="vs")
                    st[i]["v"] = vpool.tile([P, NV], F16, tag="v", name="v")
                vs_eng = nc.gpsimd if (pool_units & 4) else nc.vector
                for i in range(W):
                    pp = st[i]["pp"]
                    vs_eng.tensor_add(st[i]["vs"][:], pp[:, 0:NV],
                                      pp[:, NV : 2 * NV])
                for i in range(W):
                    nc.vector.tensor_add(st[i]["v"][:], st[i]["vs"][:],
                                         st[i]["pp"][:, 2 * NV : 3 * NV])

                # ---- masked squares: sum(relu(-v)^2) -> acc -----------------
                if stages < 4:
                    continue
                for i, g in enumerate(pair):
                    if act_relu:
                        nc.scalar.activation(
                            st[i]["vs"][:], st[i]["v"][:],
                            mybir.ActivationFunctionType.Relu, scale=-1.0)
                    else:
                        nc.vector.tensor_scalar(
                            st[i]["vs"][:], st[i]["v"][:], -1.0, 0.0,
                            mybir.AluOpType.mult, mybir.AluOpType.max)
                    nc.scalar.activation(st[i]["v"][:], st[i]["vs"][:],
                                         mybir.ActivationFunctionType.Square,
                                         accum_out=acc[:, rep * G + g : rep * G + g + 1])

                if stages < 5:
                    continue
                if not last_pair:
                    emit_cop_path()
                else:
                    # fold PSUM chunks on ACT (identity + accum); ACT reads
                    # PSUM cheaply and is otherwise idle at the tail
                    ps_scratch = vpool.tile([1, 512], F32, tag="pss", name="pss")
                    for j in range(n_chunks):
                        lo = 512 * j
                        hi = min(RED_N, lo + 512)
                        nc.scalar.activation(
                            ps_scratch[:, 0 : hi - lo],
                            psum_cop[j][:, 0 : hi - lo],
                            mybir.ActivationFunctionType.Copy,
                            accum_out=cop_acc[:, j : j + 1])

        if loop_cm is not None:
            loop_cm.__exit__(None, None, None)

        # ---- epilogue: DMA out --------------------------------------------
        nc.sync.dma_start(cop_out.ap(), cop_acc[:])
        nc.scalar.dma_start(mask_out.ap(), acc[:])

    nc.compile()
    return nc, G


def host_planarize(x: np.ndarray, n_cores: int, K: int) -> np.ndarray:
    """[B,21,3] f32 -> [cores, G, P, 60K] f16: bone layout [c][q:4][f:5][k].

    Bones are differenced in fp32 on host, then cast once to fp16.
    """
    B = x.shape[0]
    R = B // n_cores
    G = R // (P * K)
    xr = x.reshape(n_cores, G, P, K, 21, 3)
    jidx = (np.arange(5) * 4)[:, None] + np.arange(5)[None, :]  # [f, jj]
    xj = xr[:, :, :, :, jidx, :]                 # [cores,G,P,K,f,jj,3]
    bones = xj[:, :, :, :, :, 1:, :] - xj[:, :, :, :, :, :-1, :]
    xp = bones.transpose(0, 1, 2, 6, 5, 4, 3)    # [cores,G,P,c,q,f,K]
    out = np.empty((n_cores, G, P, ROW * K), dtype=np.float16)
    np.copyto(out.reshape(xp.shape), xp)
    return out


_CACHE = {}


def _get_nc(rows_per_core: int, K: int):
    key = (rows_per_core, K)
    if key not in _CACHE:
        _CACHE[key] = build_bass(rows_per_core, K)
    return _CACHE[key]


def kernel(pose23d_pred: np.ndarray) -> np.ndarray:
    x = np.asarray(pose23d_pred, dtype=np.float32)
    assert x.shape == (B_FULL, 21, 3), x.shape
    K = DEF_K
    R = B_FULL // N_CORES
    nc, G = _get_nc(R, K)
    xp = host_planarize(x, N_CORES, K)
    in_maps = [{"x": xp[i]} for i in range(N_CORES)]
    res = run_bass_kernel_spmd(nc, in_maps, list(range(N_CORES)))
    total = 0.0
    for r in res.results:
        total += r["cop_out"].astype(np.float64).sum()
        total += r["mask_out"].astype(np.float64).sum()
    return np.float32(total)
